# revision 12
# baseline (speedup 1.0000x reference)
"""DilatedAttention Trainium2 kernel (v2b).

Same math as v2 but the score computation runs in a d-transposed layout so
the d-reduction moves to the TensorEngine:

  - Q,K tiles are DMA'd in layout [(half,d)=128, (q, p')] (host pre-transposes;
    same byte count). prod1_t[p128=(h,d), q, k, p'] = Q*K on DVE (innermost p'
    64-contiguous).
  - Per q-head c: PE matmul with a [128,2] block-ones stationary reduces over
    d partitions -> scores chunk [2, g*64] in its own PSUM bank (fp32).
  - ACT applies Exp straight out of PSUM into e_t [2, g, g*64] (SBUF), one
    SBUF->SBUF DMA reshapes to the standard [128 pos, q, k] layout.
  - Softmax denominator, attn, AV product (std layout, 64-contig innermost,
    pairwise tree k-reduction) as v2. AV product engine env-switchable
    (AV_ENG=gpsimd|vector).
  - No on-device normalization or Z: host sums xn in fp64 and folds 1/(3Z)
    into the scatter-add.
"""

import sys

sys.path.insert(0, "/opt/trn_rl_repo")

import numpy as np

B, S, H, D = 4, 8192, 16, 64
NG = 3
SEG = [2048, 4096, 8192]
RATE = [1, 2, 4]
GS = [6, 5, 5]
HMIN = [0, 5, 10]
OFF = [0, 1, 2]
P = 128  # positions per tile
N6 = 32  # g6 tiles per core
N5 = 24  # g5 tiles per core
SCALE = 1.0 / 8.0

_CACHED_NC = None


def _section(nc, pools, qd, kd, vd, od, sc_ps, onesbd, ntiles, g):
    import os

    from concourse import mybir

    f32 = mybir.dt.float32
    io, work, small, singles = pools
    gd = g * D
    av_eng = (
        nc.gpsimd if os.environ.get("AV_ENG", "vector") == "gpsimd" else nc.vector
    )

    for t in range(ntiles):
        r0 = t * P
        qt_sb = io.tile([P, g, D], f32, tag=f"qt{g}")  # [(h d), (q p')]
        kt_sb = io.tile([P, g, D], f32, tag=f"kt{g}")
        v_sb = io.tile([P, g, D], f32, tag=f"v{g}")
        nc.sync.dma_start(out=qt_sb, in_=qd[r0 : r0 + P, :])
        nc.sync.dma_start(out=kt_sb, in_=kd[r0 : r0 + P, :])
        nc.sync.dma_start(out=v_sb, in_=vd[r0 : r0 + P, :])

        # prod1_t[(h d), q, k, p'] = Qt[(h d), q, p'] * Kt[(h d), k, p']
        prod1 = work.tile([P, g, g, D], f32, tag=f"prod1_{g}")
        nc.vector.tensor_mul(
            out=prod1[:],
            in0=qt_sb[:].unsqueeze(2).broadcast_to([P, g, g, D]),
            in1=kt_sb[:].unsqueeze(1).broadcast_to([P, g, g, D]),
        )

        # PE: scores chunk c = sum_d prod1[(h d), c, :, :] -> PSUM [2, (k p')]
        e_t = small.tile([2, 64, g, g], f32, tag=f"et{g}")  # [h, p', q, k]
        for c in range(g):
            nc.tensor.matmul(
                sc_ps[c][:, 0:gd],
                onesbd[:],
                prod1[:, c, :, :],
                start=True,
                stop=True,
            )
            # ACT: exp straight out of PSUM, transposing (k p') -> (p' k)
            nc.scalar.activation(
                out=e_t[:, :, c, :],
                in_=sc_ps[c][:, 0:gd].rearrange("h (k p) -> h p k", k=g),
                func=mybir.ActivationFunctionType.Exp,
            )

        # reshape [h, p', q, k] -> [pos=(h p'), q, k] via SBUF->SBUF DMA
        e_sb = small.tile([P, g, g], f32, tag=f"e{g}")
        for h in range(2):
            nc.sync.dma_start(
                out=e_sb[h * 64 : (h + 1) * 64, :, :],
                in_=e_t[h : h + 1].rearrange("h p q k -> h p (q k)"),
            )

        # den[p, q] = sum_k e ; rd = 1/den ; attn = e * rd
        den = small.tile([P, g], f32, tag=f"den{g}")
        nc.vector.tensor_reduce(
            out=den[:],
            in_=e_sb[:],
            axis=mybir.AxisListType.X,
            op=mybir.AluOpType.add,
        )
        rd = small.tile([P, g], f32, tag=f"rd{g}")
        nc.vector.reciprocal(out=rd[:], in_=den[:])
        attn = small.tile([P, g, g], f32, tag=f"attn{g}")
        nc.vector.tensor_mul(
            out=attn[:],
            in0=e_sb[:],
            in1=rd[:].unsqueeze(2).broadcast_to([P, g, g]),
        )

        # prod2[p, q, k, d] = attn[p,q,k] * V[p,k,d]
        prod2 = work.tile([P, g, g, D], f32, tag=f"prod2_{g}")
        av_eng.tensor_mul(
            out=prod2[:],
            in0=attn[:].unsqueeze(3).broadcast_to([P, g, g, D]),
            in1=v_sb[:].unsqueeze(1).broadcast_to([P, g, g, D]),
        )

        # xn[p, q, d] = sum_k prod2 (pairwise tree, 64-contig)
        xn = io.tile([P, g, D], f32, tag=f"xn{g}")
        if g == 6:
            h1 = work.tile([P, g, 3, D], f32, tag="h1_6")
            nc.vector.tensor_add(
                out=h1[:], in0=prod2[:, :, 0:3, :], in1=prod2[:, :, 3:6, :]
            )
            h2 = work.tile([P, g, 1, D], f32, tag="h2_6")
            nc.vector.tensor_add(
                out=h2[:], in0=h1[:, :, 0:1, :], in1=h1[:, :, 1:2, :]
            )
            nc.vector.tensor_add(
                out=xn[:].unsqueeze(2), in0=h2[:], in1=h1[:, :, 2:3, :]
            )
        else:
            h1 = work.tile([P, g, 2, D], f32, tag="h1_5")
            nc.vector.tensor_add(
                out=h1[:], in0=prod2[:, :, 0:2, :], in1=prod2[:, :, 2:4, :]
            )
            h2 = work.tile([P, g, 1, D], f32, tag="h2_5")
            nc.vector.tensor_add(
                out=h2[:], in0=h1[:, :, 0:1, :], in1=h1[:, :, 1:2, :]
            )
            nc.vector.tensor_add(
                out=xn[:].unsqueeze(2), in0=h2[:], in1=prod2[:, :, 4:5, :]
            )

        nc.sync.dma_start(out=od[r0 : r0 + P, :], in_=xn[:])


def _build_nc():
    import concourse.bacc as bacc
    import concourse.tile as tile
    from concourse import mybir

    f32 = mybir.dt.float32
    nc = bacc.Bacc()

    q6 = nc.dram_tensor("q6", [N6 * P, 6 * D], f32, kind="ExternalInput")
    k6 = nc.dram_tensor("k6", [N6 * P, 6 * D], f32, kind="ExternalInput")
    v6 = nc.dram_tensor("v6", [N6 * P, 6 * D], f32, kind="ExternalInput")
    q5 = nc.dram_tensor("q5", [N5 * P, 5 * D], f32, kind="ExternalInput")
    k5 = nc.dram_tensor("k5", [N5 * P, 5 * D], f32, kind="ExternalInput")
    v5 = nc.dram_tensor("v5", [N5 * P, 5 * D], f32, kind="ExternalInput")
    ob_d = nc.dram_tensor("onesbd", [P, 2], f32, kind="ExternalInput")
    o6 = nc.dram_tensor("o6", [N6 * P, 6 * D], f32, kind="ExternalOutput")
    o5 = nc.dram_tensor("o5", [N5 * P, 5 * D], f32, kind="ExternalOutput")

    with tile.TileContext(nc) as tc:
        with (
            tc.tile_pool(name="io", bufs=4) as io,
            tc.tile_pool(name="work", bufs=2) as work,
            tc.tile_pool(name="small", bufs=3) as small,
            tc.tile_pool(name="singles", bufs=1) as singles,
            tc.tile_pool(name="psum", bufs=1, space="PSUM") as psum,
        ):
            pools = (io, work, small, singles)
            onesbd = singles.tile([P, 2], f32)
            nc.sync.dma_start(out=onesbd, in_=ob_d[:, :])
            sc_ps = [
                psum.tile([2, 6 * D], f32, tag=f"sc{c}", name=f"sc{c}")
                for c in range(6)
            ]

            _section(nc, pools, q6, k6, v6, o6, sc_ps, onesbd, N6, 6)
            _section(nc, pools, q5, k5, v5, o5, sc_ps, onesbd, N5, 5)

    nc.finalize()
    return nc


def _gather(x, b, gi):
    idx = np.arange(OFF[gi], S, RATE[gi])
    return np.ascontiguousarray(x[b, idx, HMIN[gi] : HMIN[gi] + GS[gi], :])


def _transp(a, g):
    """[npos, g, 64] -> transposed tile layout [npos, g*64] with
    row = h*64+d, col = q*64+p' per 128-position tile."""
    nt = a.shape[0] // P
    # [t, h, p', q, d] -> [t, h, d, q, p']
    at = a.reshape(nt, 2, 64, g, D).transpose(0, 1, 4, 3, 2)
    return np.ascontiguousarray(at).reshape(nt * P, g * D)


def _host_pack(query, key, value):
    in_maps = []
    onesbd = np.zeros((P, 2), dtype=np.float32)
    onesbd[0:64, 0] = 1.0
    onesbd[64:128, 1] = 1.0
    for core in range(8):
        b, role = core // 2, core % 2
        qg0 = _gather(query, b, 0) * SCALE
        kg0 = _gather(key, b, 0)
        vg0 = _gather(value, b, 0)
        qg1 = _gather(query, b, 1) * SCALE
        kg1 = _gather(key, b, 1)
        vg1 = _gather(value, b, 1)
        if role == 0:
            sl6 = slice(0, N6 * P)
            qg2 = _gather(query, b, 2) * SCALE
            kg2 = _gather(key, b, 2)
            vg2 = _gather(value, b, 2)
            q5v = np.concatenate([qg2, qg1[: 8 * P]])
            k5v = np.concatenate([kg2, kg1[: 8 * P]])
            v5v = np.concatenate([vg2, vg1[: 8 * P]])
        else:
            sl6 = slice(N6 * P, 2 * N6 * P)
            q5v = qg1[8 * P : 32 * P]
            k5v = kg1[8 * P : 32 * P]
            v5v = vg1[8 * P : 32 * P]
        in_maps.append(
            {
                "q6": _transp(qg0[sl6], 6),
                "k6": _transp(kg0[sl6], 6),
                "v6": vg0[sl6].reshape(N6 * P, 6 * D),
                "q5": _transp(np.ascontiguousarray(q5v), 5),
                "k5": _transp(np.ascontiguousarray(k5v), 5),
                "v5": np.ascontiguousarray(v5v).reshape(N5 * P, 5 * D),
                "onesbd": onesbd,
            }
        )
    return in_maps


LAST_EXEC_NS = None


def kernel(query, key, value):
    global _CACHED_NC, LAST_EXEC_NS
    query = np.asarray(query, dtype=np.float32)
    key = np.asarray(key, dtype=np.float32)
    value = np.asarray(value, dtype=np.float32)

    import os

    from concourse.bass_utils import run_bass_kernel_spmd

    if _CACHED_NC is None:
        _CACHED_NC = _build_nc()
    nc = _CACHED_NC

    in_maps = _host_pack(query, key, value)
    kw = {}
    if os.environ.get("KERNEL_TRACE"):
        kw = dict(trace=True)
        tdir = os.environ.get("KERNEL_TRACE_DIR")
        if tdir:
            os.makedirs(tdir, exist_ok=True)
            kw["tmpdir"] = tdir
    try:
        res = run_bass_kernel_spmd(nc, in_maps, list(range(8)), **kw)
    except Exception:
        if not kw:
            raise
        kw = {}
        res = run_bass_kernel_spmd(nc, in_maps, list(range(8)))
    if getattr(res, "exec_time_ns", None):
        LAST_EXEC_NS = res.exec_time_ns
    results = res.results

    # ---- host: fp64 Z from device xn, fold 1/(3Z) into scatter-add ----
    xn6, xn5 = {}, {}
    Z = {}
    for b in range(B):
        for gi in range(NG):
            Z[b, gi] = np.zeros((GS[gi], D), dtype=np.float64)
    for core in range(8):
        b, role = core // 2, core % 2
        r = results[core]
        xn6[core] = np.asarray(r["o6"]).reshape(N6 * P, 6, D)
        xn5[core] = np.asarray(r["o5"]).reshape(N5 * P, 5, D)
        Z[b, 0] += np.sum(xn6[core], axis=0, dtype=np.float64)
        if role == 0:
            Z[b, 2] += np.sum(xn5[core][: 16 * P], axis=0, dtype=np.float64)
            Z[b, 1] += np.sum(xn5[core][16 * P :], axis=0, dtype=np.float64)
        else:
            Z[b, 1] += np.sum(xn5[core], axis=0, dtype=np.float64)

    out = np.zeros((B, S, H, D), dtype=np.float32)
    for b in range(B):
        rz = [(1.0 / (NG * Z[b, gi])).astype(np.float32) for gi in range(NG)]
        a_core, b_core = 2 * b, 2 * b + 1
        idx0 = np.arange(OFF[0], S, RATE[0])
        x0 = np.concatenate([xn6[a_core], xn6[b_core]])
        out[b, idx0, HMIN[0] : HMIN[0] + 6, :] += x0 * rz[0]
        idx2 = np.arange(OFF[2], S, RATE[2])
        out[b, idx2, HMIN[2] : HMIN[2] + 5, :] += xn5[a_core][: 16 * P] * rz[2]
        idx1 = np.arange(OFF[1], S, RATE[1])
        x1 = np.concatenate([xn5[a_core][16 * P :], xn5[b_core]])
        out[b, idx1, HMIN[1] : HMIN[1] + 5, :] += x1 * rz[1]
    return out
